# revision 20
# baseline (speedup 1.0000x reference)
"""Trainium2 Bass kernel for the CTRF dense_cnn problem.

y[b,t,o] = b[o] + sum_{lag in [-10,40]} sum_d W[o,(lag+10)*64+d] * x[b,t-lag,d]

Strategy (8 NeuronCores, data-parallel over batch, 2 batches/core), using a
Winograd F(4,4) decomposition of the 51-tap time conv:

  - 51 taps -> 13 groups of 4 taps (last taps zero-padded). Output tiles of
    4 timesteps (NT = 512 tiles, exactly T). Each (group, tile) contribution
    is F(4,4): 7 transform points instead of 16 tap-applications.
  - Data transform B^T and weight transform G are applied on the HOST; the
    device sees 7 pre-transformed sequences v_p and packed weights.
  - Adjacent groups (2k, 2k+1) are paired on the K dim: stationary
    [128, 128] = [U_{2k,p}; U_{2k+1,p}], moving zV_p = [v_p[c]; v_p[c-1]].
    7 pair-matmuls x 7 points accumulate M_p per 256-col chunk; the A^T
    combine + bias run as progressive scalar_tensor_tensor chains so only
    one DVE pass trails the final matmul.
  - Mixed per-point dtype: points 0..2 bf16, points 3..6 float32r
    (higher node powers amplify quantization error; f32r's ~10-bit
    mantissa keeps total rel err ~6e-3 « 2e-2 while bf16 halves DMA
    for the benign points).
  - PE cols per core: 2 * 7 * 7 * 512 = 50,176 vs 106,496 direct (2.12x).
"""

import os
import sys

os.environ.setdefault("MYCRO_LOCAL_CACHE", "1")

for _p in ("/opt/trn_rl_repo", "/root/.axon_site/_ro/trn_rl_repo"):
    if os.path.isdir(_p) and _p not in sys.path:
        sys.path.insert(0, _p)

import ml_dtypes
import numpy as np

import concourse.mybir as mybir  # noqa: E402
import concourse.tile as tile  # noqa: E402
from concourse import bacc  # noqa: E402
from concourse.bass_utils import run_bass_kernel_spmd  # noqa: E402

N_CORES = 8
B, T, D, O = 16, 2048, 64, 128
NLAGS = 51
BPC = B // N_CORES  # batches per core
M_TILE = 4          # outputs per tile
NPTS = 7            # transform points
NT = T // M_TILE    # 512 output tiles per batch
NG = 13             # tap groups of 4 (taps 51 zero-padded)
NPAIR = 7           # group pairs on K (pair 6 bottom half zero)
C0 = 13             # col offset: c = tau - g + C0
XBASE = 11 - M_TILE - M_TILE * C0  # window origin: x[4c + XBASE + s]
ZC = 526            # zV cols (cc 1..524 used)
CW = 512            # chunk width: one chunk per batch (7 banks + warm)
# matmul/point order: bf16 points first to build DMA slack for f32r points
POINT_ORDER = (0, 1, 2, 5, 3, 4, 6)
N_WARM = 6          # f32 warm matmuls to open the HAM clock gate

# per-point dtype: low-power nodes tolerate bf16; high-power need f32r
PT_BF16 = (True, True, True, False, False, False, False)

# ---- F(4,4) transform matrices, nodes [0, 1, -1, 2, -2, 1/2] + inf -------
_nodes = [0.0, 1.0, -1.0, 2.0, -2.0, 0.5]
_E = np.zeros((7, 4))
for _i, _a in enumerate(_nodes):
    _E[_i] = [_a**_k for _k in range(4)]
_E[6, 3] = 1.0
G_MAT = _E  # filter degree 3 -> same evaluation matrix
_V = np.zeros((7, 7))
for _i, _a in enumerate(_nodes):
    _V[_i] = [_a**_k for _k in range(7)]
_V[6, 6] = 1.0
BT_MAT = np.linalg.inv(_V).T
# A^T = E^T:
#   y0 = M0+M1+M2+M3+M4+M5
#   y1 = M1-M2+2M3-2M4+.5M5
#   y2 = M1+M2+4M3+4M4+.25M5
#   y3 = M1-M2+8M3-8M4+.125M5+M6


def _build():
    nc = bacc.Bacc(
        "TRN2", target_bir_lowering=False, debug=False, num_devices=N_CORES
    )
    f32 = mybir.dt.float32
    bf16 = mybir.dt.bfloat16
    f32r = mybir.dt.float32r
    Alu = mybir.AluOpType
    pt_dt = [bf16 if b_ else f32r for b_ in PT_BF16]

    zv_ds = [
        nc.declare_dram_parameter(f"zv{p}", [BPC, 128, ZC], pt_dt[p], isOutput=False)
        for p in range(NPTS)
    ]
    wg_ds = [
        nc.declare_dram_parameter(f"wg{p}", [128, NPAIR, O], pt_dt[p], isOutput=False)
        for p in range(NPTS)
    ]
    b_d = nc.declare_dram_parameter("bvec", [O, 1], f32, isOutput=False)
    y_d = nc.declare_dram_parameter("y", [BPC, M_TILE, O, NT], f32, isOutput=True)

    with tile.TileContext(nc) as tc:
        with (
            tc.tile_pool(name="consts", bufs=1) as consts,
            tc.tile_pool(name="zv", bufs=1) as zv_pool,
            tc.tile_pool(name="csb", bufs=2) as csb_pool,
            tc.tile_pool(name="ysb", bufs=2) as ysb_pool,
            tc.tile_pool(name="pacc", bufs=8, space="PSUM") as pacc_pool,
        ):
            # HAM warmup (PE clock gate opens after a few us of activity).
            wsrc = consts.tile([128, 128], f32, tag="wsrc")
            nc.vector.memset(wsrc[:], 1.0)
            warm_ps = pacc_pool.tile([128, CW], f32, tag="pacc")
            for _ in range(N_WARM):
                nc.tensor.matmul(
                    warm_ps[:, 0:128], wsrc[:], wsrc[:], start=True, stop=True
                )

            # Input DMAs in consumption order (POINT_ORDER, batch-major).
            bias_sb = consts.tile([O, 1], f32)
            wg_sbs = [None] * NPTS
            zv_sbs = [[None] * NPTS for _ in range(BPC)]
            first = True
            for p in POINT_ORDER:
                wg_sb = consts.tile([128, NPAIR, O], pt_dt[p], tag=f"wg{p}")
                nc.sync.dma_start(wg_sb[:], wg_ds[p][:])
                wg_sbs[p] = wg_sb
                zt = zv_pool.tile([128, ZC], pt_dt[p], tag=f"zv0_{p}")
                zv_sbs[0][p] = zt
                nc.sync.dma_start(zt[:], zv_ds[p][0])
                if first:
                    nc.sync.dma_start(bias_sb[:], b_d[:])
                    first = False
            for bb in range(1, BPC):
                for p in POINT_ORDER:
                    zt = zv_pool.tile([128, ZC], pt_dt[p], tag=f"zv{bb}_{p}")
                    zv_sbs[bb][p] = zt
                    nc.sync.dma_start(zt[:], zv_ds[p][bb])

            def ctile(tag):
                return csb_pool.tile([128, CW], f32, tag=tag, name=tag)

            def stt(out, psum, scalar, sbuf, op0):
                nc.vector.scalar_tensor_tensor(
                    out[:, 0:CW], psum[:, 0:CW], scalar, sbuf[:, 0:CW],
                    op0, Alu.add,
                )

            def emit_chunk(bb, t0):
                Ident = mybir.ActivationFunctionType.Identity
                M = {}
                st = {}
                a = None
                for p in POINT_ORDER:
                    pacc = pacc_pool.tile([128, CW], f32, tag="pacc")
                    M[p] = pacc
                    for k in range(NPAIR):
                        off = C0 - 2 * k
                        nc.tensor.matmul(
                            pacc[:],
                            wg_sbs[p][:, k, :],
                            zv_sbs[bb][p][:, t0 + off : t0 + off + CW],
                            start=(k == 0),
                            stop=(k == NPAIR - 1),
                        )
                    # A^T combine split across engines:
                    #  Scalar: scaled/biased PSUM->SBUF copies of M1..M5
                    #  Vector: stt chains (each reads <=1 PSUM: M0/M2/M3/M4/M5/M6)
                    #  GpSimd: SBUF-only tensor_tensor adds/subs + tensor_scalar
                    if p == 1:
                        st["m1b"] = ctile("m1b")
                        nc.scalar.activation(st["m1b"][:], M[1][:], Ident, bias=bias_sb[:])
                        a = ctile("a")
                        stt(a, M[0], 0.0, st["m1b"], Alu.add)
                    elif p == 2:
                        st["m2"] = ctile("m2")
                        nc.scalar.activation(st["m2"][:], M[2][:], Ident)
                        a2 = ctile("a")
                        stt(a2, M[2], 0.0, a, Alu.add)
                        a = a2
                        st["dd"] = ctile("dd")
                        nc.gpsimd.tensor_tensor(st["dd"][:], st["m1b"][:], st["m2"][:], Alu.subtract)
                        st["t"] = ctile("t")
                        nc.gpsimd.tensor_tensor(st["t"][:], st["m1b"][:], st["m2"][:], Alu.add)
                    elif p == 5:
                        st["m5h"] = ctile("m5h")
                        nc.scalar.activation(st["m5h"][:], M[5][:], Ident, scale=0.5)
                        st["m5q"] = ctile("m5q")
                        nc.scalar.activation(st["m5q"][:], M[5][:], Ident, scale=0.25)
                        st["m5e"] = ctile("m5e")
                        nc.scalar.activation(st["m5e"][:], M[5][:], Ident, scale=0.125)
                        a2 = ctile("a")
                        stt(a2, M[5], 0.0, a, Alu.add)
                        a = a2
                        st["d5"] = ctile("d5")
                        nc.gpsimd.tensor_tensor(st["d5"][:], st["dd"][:], st["m5h"][:], Alu.add)
                        st["w5"] = ctile("w5")
                        nc.gpsimd.tensor_tensor(st["w5"][:], st["dd"][:], st["m5e"][:], Alu.add)
                        st["t5"] = ctile("t5")
                        nc.vector.tensor_tensor(st["t5"][:], st["t"][:], st["m5q"][:], Alu.add)
                    elif p == 3:
                        st["m3d"] = ctile("m3d")
                        nc.scalar.activation(st["m3d"][:], M[3][:], Ident, scale=2.0)
                        a2 = ctile("a")
                        stt(a2, M[3], 0.0, a, Alu.add)
                        a = a2
                        st["u"] = ctile("u")
                        nc.gpsimd.tensor_tensor(st["u"][:], st["d5"][:], st["m3d"][:], Alu.add)
                        st["m3q"] = ctile("m3q")
                        nc.gpsimd.tensor_scalar_mul(st["m3q"][:], st["m3d"][:], 4.0)
                        st["w"] = ctile("w")
                        nc.gpsimd.tensor_tensor(st["w"][:], st["w5"][:], st["m3q"][:], Alu.add)
                        st["v"] = ctile("v")
                        nc.vector.scalar_tensor_tensor(
                            st["v"][:], st["m3d"][:], 2.0, st["t5"][:], Alu.mult, Alu.add
                        )
                    elif p == 4:
                        st["m4d"] = ctile("m4d")
                        nc.scalar.activation(st["m4d"][:], M[4][:], Ident, scale=2.0)
                        y0 = ysb_pool.tile([128, CW], f32, tag="y0")
                        stt(y0, M[4], 0.0, a, Alu.add)
                        nc.sync.dma_start(y_d[bb, 0, :, t0 : t0 + CW], y0[:])
                        y1 = ysb_pool.tile([128, CW], f32, tag="y1")
                        nc.gpsimd.tensor_tensor(y1[:], st["u"][:], st["m4d"][:], Alu.subtract)
                        nc.sync.dma_start(y_d[bb, 1, :, t0 : t0 + CW], y1[:])
                        st["m4q"] = ctile("m4q")
                        nc.gpsimd.tensor_scalar_mul(st["m4q"][:], st["m4d"][:], 4.0)
                        st["w2"] = ctile("w2")
                        nc.gpsimd.tensor_tensor(st["w2"][:], st["w"][:], st["m4q"][:], Alu.subtract)
                        y2 = ysb_pool.tile([128, CW], f32, tag="y2")
                        nc.vector.scalar_tensor_tensor(
                            y2[:], st["m4d"][:], 2.0, st["v"][:], Alu.mult, Alu.add
                        )
                        nc.sync.dma_start(y_d[bb, 2, :, t0 : t0 + CW], y2[:])
                # y3 = M6 + w2 — the only pass after the final matmul
                y3 = ysb_pool.tile([128, CW], f32, tag="y3")
                stt(y3, M[6], 0.0, st["w2"], Alu.add)
                nc.sync.dma_start(y_d[bb, 3, :, t0 : t0 + CW], y3[:])

            for bb in range(BPC):
                for t0 in range(0, NT, CW):
                    emit_chunk(bb, t0)
    nc.compile()
    return nc


_NC_CACHE = {}


def _get_program():
    if "nc" not in _NC_CACHE:
        _NC_CACHE["nc"] = _build()
    return _NC_CACHE["nc"]


def _prep_inputs(x, W, b):
    x = np.ascontiguousarray(x, dtype=np.float32)
    W = np.ascontiguousarray(W, dtype=np.float32)
    b = np.ascontiguousarray(b, dtype=np.float32)

    # --- data transform: v_p[c] = sum_s BT[p,s] x[4c + XBASE + s] ---------
    PAD = 80
    xpad = np.zeros((B, T + 2 * PAD, D), dtype=np.float32)
    xpad[:, PAD : PAD + T] = x
    cs = np.arange(-1, ZC)  # c for top cols 0..ZC-1, bottom needs c-1 >= -1
    idx = PAD + M_TILE * cs[:, None] + XBASE + np.arange(NPTS)[None, :]
    dwin = xpad[:, idx]                          # [B, nc, 7, D]
    v = np.einsum(
        "ps,bcsd->bpcd", BT_MAT.astype(np.float32), dwin
    )                                            # [B, 7, nc, D]
    vt = v.transpose(0, 1, 3, 2)                 # [B, 7, D, nc]; col k = c+1
    zv_f32 = np.zeros((B, NPTS, 128, ZC), dtype=np.float32)
    zv_f32[:, :, :D, :] = vt[:, :, :, 1 : 1 + ZC]
    zv_f32[:, :, D:, :] = vt[:, :, :, 0:ZC]

    # --- weight transform -------------------------------------------------
    # U_{g,p}[d, o] = sum_i G[p,i] * W[o, (4g+3-i)*64 + d], tap >= 51 -> 0
    Wblk = W.reshape(O, NLAGS, D)
    wg = np.zeros((NPTS, 128, NPAIR, O), dtype=np.float32)
    for p in range(NPTS):
        for g in range(NG):
            U = np.zeros((D, O), dtype=np.float32)
            for i in range(M_TILE):
                tap = M_TILE * g + M_TILE - 1 - i
                if tap < NLAGS:
                    U += np.float32(G_MAT[p, i]) * Wblk[:, tap, :].T
            k, half = divmod(g, 2)
            wg[p, half * D : (half + 1) * D, k, :] = U

    def cast(a, p):
        a = np.ascontiguousarray(a)
        return a.astype(ml_dtypes.bfloat16) if PT_BF16[p] else a

    wg_maps = {f"wg{p}": cast(wg[p], p) for p in range(NPTS)}
    bvec = np.ascontiguousarray(b.reshape(O, 1))
    maps = []
    for c in range(N_CORES):
        m = {"bvec": bvec}
        for p in range(NPTS):
            m[f"zv{p}"] = cast(zv_f32[c * BPC : (c + 1) * BPC, p], p)
        m.update(wg_maps)
        maps.append(m)
    return maps


def _assemble(res):
    # Per core: y_raw [BPC, 4, O, NT]; y[b, 4*tau+r, o] = y_raw[b, r, o, tau]
    outs = []
    for c in range(N_CORES):
        y_raw = res.results[c]["y"]
        y = (
            y_raw.transpose(0, 3, 1, 2)
            .reshape(BPC, T, O)
            .astype(np.float32)
        )
        outs.append(np.ascontiguousarray(y))
    return np.concatenate(outs, axis=0)


def kernel(x, W, b):
    in_maps = _prep_inputs(x, W, b)
    res = run_bass_kernel_spmd(
        _get_program(), in_maps, core_ids=list(range(N_CORES))
    )
    return _assemble(res)


def _ensure_ntff_hook():
    """The agent image's antenv lacks axon_hooks, so run_bass_kernel_spmd's
    trace path degrades to no-profile. Seed an equivalent module backed by
    the ctypes NTFF profiler from trn_agent_boot."""
    try:
        from antenv.axon_hooks import get_axon_ntff_profile_hook

        if get_axon_ntff_profile_hook() is not None:
            return True
    except ImportError:
        pass
    try:
        import types

        site_dir = "/root/.axon_site"
        if site_dir not in sys.path and os.path.isdir(site_dir):
            sys.path.insert(0, site_dir)
        from trn_agent_boot.trn_boot import _ntff_profile_via_ctypes

        hook = _ntff_profile_via_ctypes("/opt/axon/libaxon_pjrt.so")
        if hook is None:
            return False
        mod = types.ModuleType("antenv.axon_hooks")
        mod.get_axon_ntff_profile_hook = lambda: hook
        mod.set_axon_ntff_profile_hook = lambda h: None
        sys.modules["antenv.axon_hooks"] = mod
        import antenv

        antenv.axon_hooks = mod
        return True
    except Exception:
        return False


def kernel_traced(x, W, b, **kwargs):
    """Like kernel() but requests an NTFF trace; returns (y, BassKernelResults).

    Dev-loop only (test.py); the graded kernel() path never traces. The
    artifact upload is stubbed out since this container has no bucket access.
    """
    _ensure_ntff_hook()
    from concourse import bass_utils as _bu

    in_maps = _prep_inputs(x, W, b)
    orig_upload = _bu.upload_artifacts
    _bu.upload_artifacts = lambda tmpdir: f"local:{tmpdir}"
    try:
        res = run_bass_kernel_spmd(
            _get_program(), in_maps, core_ids=list(range(N_CORES)), trace=True, **kwargs
        )
    finally:
        _bu.upload_artifacts = orig_upload
    y = _assemble(res)
    return y, res


# revision 21
# speedup vs baseline: 1.9371x; 1.9371x over previous
"""Trainium2 Bass kernel for the CTRF dense_cnn problem.

y[b,t,o] = b[o] + sum_{lag in [-10,40]} sum_d W[o,(lag+10)*64+d] * x[b,t-lag,d]

Strategy (8 NeuronCores, data-parallel over batch, 2 batches/core), using a
Winograd F(3,3) decomposition of the 51-tap time conv:

  - 51 taps -> 18 groups of 3 taps (last group zero-padded). Output tiles of
    3 timesteps. Each (group, tile) contribution is F(3,3): 5 transform
    points instead of 9 tap-applications.
  - The data transform B^T d is applied on the HOST (it's a cheap linear map
    over x); the device sees 5 pre-transformed sequences v_p. The weight
    transform (G applied to reversed tap blocks) is also host-side.
  - Adjacent groups (2k, 2k+1) are paired on the K dim: stationary
    [128, 128] = [U_{2k,p}; U_{2k+1,p}], moving zV_p = [v_p[c]; v_p[c-1]].
    9 pair-matmuls x 5 points accumulate M_p per output-tile chunk; a short
    scalar_tensor_tensor combine applies A^T and the bias.
  - PE cols per core: 2 * 5 * 9 * 683 = 61,470 vs 106,496 direct (1.73x).

Everything is shipped bf16 (PE rate is the same as f32r; half the DMA);
host does all transforms/layout in f32; validated rel err ~7e-3 « 2e-2.
"""

import os
import sys

os.environ.setdefault("MYCRO_LOCAL_CACHE", "1")

for _p in ("/opt/trn_rl_repo", "/root/.axon_site/_ro/trn_rl_repo"):
    if os.path.isdir(_p) and _p not in sys.path:
        sys.path.insert(0, _p)

import ml_dtypes
import numpy as np

import concourse.mybir as mybir  # noqa: E402
import concourse.tile as tile  # noqa: E402
from concourse import bacc  # noqa: E402
from concourse.bass_utils import run_bass_kernel_spmd  # noqa: E402

N_CORES = 8
B, T, D, O = 16, 2048, 64, 128
NLAGS = 51
BPC = B // N_CORES  # batches per core
NT = 683            # output tiles of 3 (683*3 = 2049, last output trimmed)
NG = 18             # tap groups of 3 (taps 51..53 zero)
NPAIR = 9
PADL = 15           # zV col cc = c + PADL, c in [-15, 684]
ZC = 702            # zV cols (cc 0..699 used, +2 slack)
ZSPLIT = 544        # head/tail split of zV DMA (chunk0 reads cols < 544)
CHUNKS = [(0, 512), (512, 171)]
N_WARM = 8          # f32 warm matmuls to open the HAM clock gate

# ---- F(3,3) transform matrices, nodes [0, 1, -1, 2] + inf ----------------
_nodes = [0.0, 1.0, -1.0, 2.0]
_E = np.zeros((5, 3))
for _i, _a in enumerate(_nodes):
    _E[_i] = [1.0, _a, _a * _a]
_E[4] = [0.0, 0.0, 1.0]
G_MAT = _E  # weight transform (filter degree 2)
_V = np.zeros((5, 5))
for _i, _a in enumerate(_nodes):
    _V[_i] = [_a**_k for _k in range(5)]
_V[4] = [0, 0, 0, 0, 1]
BT_MAT = np.linalg.inv(_V).T  # data transform: v = BT @ window
# A^T = E^T = [[1,1,1,1,0],[0,1,-1,2,0],[0,1,1,4,1]]:
#   y0 = M0+M1+M2+M3, y1 = M1-M2+2*M3, y2 = M1+M2+4*M3+M4


def _build():
    nc = bacc.Bacc(
        "TRN2", target_bir_lowering=False, debug=False, num_devices=N_CORES
    )
    f32 = mybir.dt.float32
    bf16 = mybir.dt.bfloat16
    Alu = mybir.AluOpType

    zv_ds = [
        nc.declare_dram_parameter(f"zv{p}", [BPC, 128, ZC], bf16, isOutput=False)
        for p in range(5)
    ]
    wg_ds = [
        nc.declare_dram_parameter(f"wg{p}", [128, NPAIR, O], bf16, isOutput=False)
        for p in range(5)
    ]
    b_d = nc.declare_dram_parameter("bvec", [O, 1], f32, isOutput=False)
    y_d = nc.declare_dram_parameter("y", [BPC, 3, O, NT], f32, isOutput=True)

    with tile.TileContext(nc) as tc:
        with (
            tc.tile_pool(name="consts", bufs=1) as consts,
            tc.tile_pool(name="zv", bufs=1) as zv_pool,
            tc.tile_pool(name="csb", bufs=2) as csb_pool,
            tc.tile_pool(name="ysb", bufs=2) as ysb_pool,
            tc.tile_pool(name="pacc", bufs=8, space="PSUM") as pacc_pool,
        ):
            # HAM warmup (PE clock gate opens after ~5us of matmul activity).
            wsrc = consts.tile([128, 128], f32, tag="wsrc")
            nc.vector.memset(wsrc[:], 1.0)
            warm_ps = pacc_pool.tile([128, 512], f32, tag="pacc")
            for _ in range(N_WARM):
                nc.tensor.matmul(
                    warm_ps[:, 0:128], wsrc[:], wsrc[:], start=True, stop=True
                )

            # Input DMAs in consumption order.
            bias_sb = consts.tile([O, 1], f32)
            wg_sbs = []
            zv_sbs = [[None] * 5 for _ in range(BPC)]
            for p in range(5):
                wg_sb = consts.tile([128, NPAIR, O], bf16, tag=f"wg{p}")
                nc.sync.dma_start(wg_sb[:], wg_ds[p][:])
                wg_sbs.append(wg_sb)
                zt = zv_pool.tile([128, ZC], bf16, tag=f"zv0_{p}")
                zv_sbs[0][p] = zt
                nc.sync.dma_start(zt[:, 0:ZSPLIT], zv_ds[p][0, :, 0:ZSPLIT])
                if p == 0:
                    nc.sync.dma_start(bias_sb[:], b_d[:])
            for p in range(5):
                nc.sync.dma_start(
                    zv_sbs[0][p][:, ZSPLIT:], zv_ds[p][0, :, ZSPLIT:]
                )
            for bb in range(1, BPC):
                for p in range(5):
                    zt = zv_pool.tile([128, ZC], bf16, tag=f"zv{bb}_{p}")
                    zv_sbs[bb][p] = zt
                    nc.sync.dma_start(zt[:, 0:ZSPLIT], zv_ds[p][bb, :, 0:ZSPLIT])
                for p in range(5):
                    nc.sync.dma_start(
                        zv_sbs[bb][p][:, ZSPLIT:], zv_ds[p][bb, :, ZSPLIT:]
                    )

            def emit_chunk(bb, t0, cw):
                M = []
                for p in range(5):
                    pacc = pacc_pool.tile([128, 512], f32, tag="pacc")
                    M.append(pacc)
                    for k in range(NPAIR):
                        off = 17 - 2 * k
                        nc.tensor.matmul(
                            pacc[:, 0:cw],
                            wg_sbs[p][:, k, :],
                            zv_sbs[bb][p][:, t0 + off : t0 + off + cw],
                            start=(k == 0),
                            stop=(k == NPAIR - 1),
                        )
                    # DVE reads at most ONE PSUM operand per op: chain through
                    # SBUF intermediates, one M_p per pass.
                    if p == 1:
                        # t = M1 + bias
                        tt = csb_pool.tile([128, 512], f32, tag="t")
                        nc.vector.tensor_scalar_add(
                            tt[:, 0:cw], M[1][:, 0:cw], bias_sb[:]
                        )
                    if p == 2:
                        # t2 = M2 + t = M1 + M2 + bias
                        t2 = csb_pool.tile([128, 512], f32, tag="t2")
                        nc.vector.scalar_tensor_tensor(
                            t2[:, 0:cw], M[2][:, 0:cw], 0.0,
                            tt[:, 0:cw], Alu.add, Alu.add,
                        )
                    if p == 3:
                        # y0 = M0 + M3 + t2
                        s0 = csb_pool.tile([128, 512], f32, tag="s0")
                        nc.vector.scalar_tensor_tensor(
                            s0[:, 0:cw], M[0][:, 0:cw], 0.0,
                            t2[:, 0:cw], Alu.add, Alu.add,
                        )
                        y0 = ysb_pool.tile([128, 512], f32, tag="y0")
                        nc.vector.scalar_tensor_tensor(
                            y0[:, 0:cw], M[3][:, 0:cw], 0.0,
                            s0[:, 0:cw], Alu.add, Alu.add,
                        )
                        nc.sync.dma_start(
                            y_d[bb, 0, :, t0 : t0 + cw], y0[:, 0:cw]
                        )
                        # y1 = -2*M2 + t2 + 2*M3
                        u = csb_pool.tile([128, 512], f32, tag="u")
                        nc.vector.scalar_tensor_tensor(
                            u[:, 0:cw], M[2][:, 0:cw], -2.0,
                            t2[:, 0:cw], Alu.mult, Alu.add,
                        )
                        y1 = ysb_pool.tile([128, 512], f32, tag="y1")
                        nc.vector.scalar_tensor_tensor(
                            y1[:, 0:cw], M[3][:, 0:cw], 2.0,
                            u[:, 0:cw], Alu.mult, Alu.add,
                        )
                        nc.sync.dma_start(
                            y_d[bb, 1, :, t0 : t0 + cw], y1[:, 0:cw]
                        )
                        # v = 4*M3 + t2 (y2 minus M4)
                        vv = csb_pool.tile([128, 512], f32, tag="v")
                        nc.vector.scalar_tensor_tensor(
                            vv[:, 0:cw], M[3][:, 0:cw], 4.0,
                            t2[:, 0:cw], Alu.mult, Alu.add,
                        )
                # y2 = M4 + v — the only combine pass after the last matmul.
                y2 = ysb_pool.tile([128, 512], f32, tag="y2")
                nc.vector.scalar_tensor_tensor(
                    y2[:, 0:cw], M[4][:, 0:cw], 0.0,
                    vv[:, 0:cw], Alu.add, Alu.add,
                )
                nc.sync.dma_start(y_d[bb, 2, :, t0 : t0 + cw], y2[:, 0:cw])

            for bb in range(BPC):
                for t0, cw in CHUNKS:
                    emit_chunk(bb, t0, cw)
    nc.compile()
    return nc


_NC_CACHE = {}


def _get_program():
    if "nc" not in _NC_CACHE:
        _NC_CACHE["nc"] = _build()
    return _NC_CACHE["nc"]


def _prep_inputs(x, W, b):
    x = np.ascontiguousarray(x, dtype=np.float32)
    W = np.ascontiguousarray(W, dtype=np.float32)
    b = np.ascontiguousarray(b, dtype=np.float32)

    # --- data transform: v_p[c] = sum_s BT[p,s] x[3c + 2 + s] -------------
    # window origin for col c is 3c+2; c in [-16, 685] to cover tops/bottoms.
    xpad = np.zeros((B, T + 120, D), dtype=np.float32)
    xpad[:, 60 : 60 + T] = x
    cs = np.arange(-16, 686)
    idx = 60 + 3 * cs[None, :, None] + 2 + np.arange(5)[None, None, :]
    dwin = xpad[:, idx[0]]                       # [B, nc, 5, D]
    v = np.einsum(
        "ps,bcsd->bpcd", BT_MAT.astype(np.float32), dwin
    )                                            # [B, 5, nc, D]
    # zV[b, p, part, cc]: top v_p[cc-15], bottom v_p[cc-16]; cs[k]=c -> k=c+16
    zv = np.zeros((B, 5, 128, ZC), dtype=ml_dtypes.bfloat16)
    # cc in [0, 699]: top k = cc+1, bottom k = cc
    vt = v.transpose(0, 1, 3, 2)                 # [B, 5, D, nc]
    zv[:, :, :D, 0:700] = vt[:, :, :, 1:701]
    zv[:, :, D:, 0:700] = vt[:, :, :, 0:700]

    # --- weight transform -------------------------------------------------
    # U_{g,p}[d, o] = sum_i G[p,i] * W[o, (3g+2-i)*64 + d], tap >= 51 -> 0
    Wblk = W.reshape(O, NLAGS, D)
    wg = np.zeros((5, 128, NPAIR, O), dtype=np.float32)
    for p in range(5):
        for g in range(NG):
            U = np.zeros((D, O), dtype=np.float32)
            for i in range(3):
                tap = 3 * g + 2 - i
                if tap < NLAGS:
                    U += G_MAT[p, i].astype(np.float32) * Wblk[:, tap, :].T
            k, half = divmod(g, 2)
            wg[p, half * D : (half + 1) * D, k, :] = U
    wg_maps = {
        f"wg{p}": np.ascontiguousarray(wg[p]).astype(ml_dtypes.bfloat16)
        for p in range(5)
    }
    bvec = np.ascontiguousarray(b.reshape(O, 1))
    maps = []
    for c in range(N_CORES):
        m = {"bvec": bvec}
        for p in range(5):
            m[f"zv{p}"] = np.ascontiguousarray(
                zv[c * BPC : (c + 1) * BPC, p]
            )
        m.update(wg_maps)
        maps.append(m)
    return maps


def _assemble(res):
    # Per core: y_raw [BPC, 3, O, NT]; y[b, 3*tau+r, o] = y_raw[b, r, o, tau]
    outs = []
    for c in range(N_CORES):
        y_raw = res.results[c]["y"]
        y = (
            y_raw.transpose(0, 3, 1, 2)
            .reshape(BPC, NT * 3, O)[:, :T]
            .astype(np.float32)
        )
        outs.append(np.ascontiguousarray(y))
    return np.concatenate(outs, axis=0)


def kernel(x, W, b):
    in_maps = _prep_inputs(x, W, b)
    res = run_bass_kernel_spmd(
        _get_program(), in_maps, core_ids=list(range(N_CORES))
    )
    return _assemble(res)


def _ensure_ntff_hook():
    """The agent image's antenv lacks axon_hooks, so run_bass_kernel_spmd's
    trace path degrades to no-profile. Seed an equivalent module backed by
    the ctypes NTFF profiler from trn_agent_boot."""
    try:
        from antenv.axon_hooks import get_axon_ntff_profile_hook

        if get_axon_ntff_profile_hook() is not None:
            return True
    except ImportError:
        pass
    try:
        import types

        site_dir = "/root/.axon_site"
        if site_dir not in sys.path and os.path.isdir(site_dir):
            sys.path.insert(0, site_dir)
        from trn_agent_boot.trn_boot import _ntff_profile_via_ctypes

        hook = _ntff_profile_via_ctypes("/opt/axon/libaxon_pjrt.so")
        if hook is None:
            return False
        mod = types.ModuleType("antenv.axon_hooks")
        mod.get_axon_ntff_profile_hook = lambda: hook
        mod.set_axon_ntff_profile_hook = lambda h: None
        sys.modules["antenv.axon_hooks"] = mod
        import antenv

        antenv.axon_hooks = mod
        return True
    except Exception:
        return False


def kernel_traced(x, W, b, **kwargs):
    """Like kernel() but requests an NTFF trace; returns (y, BassKernelResults).

    Dev-loop only (test.py); the graded kernel() path never traces. The
    artifact upload is stubbed out since this container has no bucket access.
    """
    _ensure_ntff_hook()
    from concourse import bass_utils as _bu

    in_maps = _prep_inputs(x, W, b)
    orig_upload = _bu.upload_artifacts
    _bu.upload_artifacts = lambda tmpdir: f"local:{tmpdir}"
    try:
        res = run_bass_kernel_spmd(
            _get_program(), in_maps, core_ids=list(range(N_CORES)), trace=True, **kwargs
        )
    finally:
        _bu.upload_artifacts = orig_upload
    y = _assemble(res)
    return y, res


# revision 22
# speedup vs baseline: 1.9754x; 1.0198x over previous
"""Trainium2 Bass kernel for the CTRF dense_cnn problem.

y[b,t,o] = b[o] + sum_{lag in [-10,40]} sum_d W[o,(lag+10)*64+d] * x[b,t-lag,d]

Strategy (8 NeuronCores, data-parallel over batch, 2 batches/core), using a
Winograd F(3,3) decomposition of the 51-tap time conv:

  - 51 taps -> 18 groups of 3 taps (last group zero-padded). Output tiles of
    3 timesteps. Each (group, tile) contribution is F(3,3): 5 transform
    points instead of 9 tap-applications.
  - The data transform B^T d is applied on the HOST (it's a cheap linear map
    over x); the device sees 5 pre-transformed sequences v_p. The weight
    transform (G applied to reversed tap blocks) is also host-side.
  - Adjacent groups (2k, 2k+1) are paired on the K dim: stationary
    [128, 128] = [U_{2k,p}; U_{2k+1,p}], moving zV_p = [v_p[c]; v_p[c-1]].
    9 pair-matmuls x 5 points accumulate M_p per output-tile chunk; a short
    scalar_tensor_tensor combine applies A^T and the bias.
  - PE cols per core: 2 * 5 * 9 * 683 = 61,470 vs 106,496 direct (1.73x).

Everything is shipped bf16 (PE rate is the same as f32r; half the DMA);
host does all transforms/layout in f32; validated rel err ~7e-3 « 2e-2.
"""

import os
import sys

os.environ.setdefault("MYCRO_LOCAL_CACHE", "1")

for _p in ("/opt/trn_rl_repo", "/root/.axon_site/_ro/trn_rl_repo"):
    if os.path.isdir(_p) and _p not in sys.path:
        sys.path.insert(0, _p)

import ml_dtypes
import numpy as np

import concourse.mybir as mybir  # noqa: E402
import concourse.tile as tile  # noqa: E402
from concourse import bacc  # noqa: E402
from concourse.bass_utils import run_bass_kernel_spmd  # noqa: E402

N_CORES = 8
B, T, D, O = 16, 2048, 64, 128
NLAGS = 51
BPC = B // N_CORES  # batches per core
NT = 683            # output tiles of 3 (683*3 = 2049, last output trimmed)
NG = 18             # tap groups of 3 (taps 51..53 zero)
NPAIR = 9
PADL = 15           # zV col cc = c + PADL, c in [-15, 684]
ZC = 702            # zV cols (cc 0..699 used, +2 slack)
ZSPLIT = 544        # head/tail split of zV DMA (chunk0 reads cols < 544)
CHUNKS = [(0, 512), (512, 171)]
N_WARM = 8          # f32 warm matmuls to open the HAM clock gate

# ---- F(3,3) transform matrices, nodes [0, 1, -1, 2] + inf ----------------
_nodes = [0.0, 1.0, -1.0, 2.0]
_E = np.zeros((5, 3))
for _i, _a in enumerate(_nodes):
    _E[_i] = [1.0, _a, _a * _a]
_E[4] = [0.0, 0.0, 1.0]
G_MAT = _E  # weight transform (filter degree 2)
_V = np.zeros((5, 5))
for _i, _a in enumerate(_nodes):
    _V[_i] = [_a**_k for _k in range(5)]
_V[4] = [0, 0, 0, 0, 1]
BT_MAT = np.linalg.inv(_V).T  # data transform: v = BT @ window
# A^T = E^T = [[1,1,1,1,0],[0,1,-1,2,0],[0,1,1,4,1]]:
#   y0 = M0+M1+M2+M3, y1 = M1-M2+2*M3, y2 = M1+M2+4*M3+M4


def _build():
    nc = bacc.Bacc(
        "TRN2", target_bir_lowering=False, debug=False, num_devices=N_CORES
    )
    f32 = mybir.dt.float32
    bf16 = mybir.dt.bfloat16
    Alu = mybir.AluOpType

    zv_ds = [
        nc.declare_dram_parameter(f"zv{p}", [BPC, 128, ZC], bf16, isOutput=False)
        for p in range(5)
    ]
    wg_ds = [
        nc.declare_dram_parameter(f"wg{p}", [128, NPAIR, O], bf16, isOutput=False)
        for p in range(5)
    ]
    wg0a_d = nc.declare_dram_parameter("wg0a", [128, 2, O], bf16, isOutput=False)
    b_d = nc.declare_dram_parameter("bvec", [O, 1], f32, isOutput=False)
    y_d = nc.declare_dram_parameter("y", [BPC, 3, O, NT], f32, isOutput=True)

    with tile.TileContext(nc) as tc:
        with (
            tc.tile_pool(name="consts", bufs=1) as consts,
            tc.tile_pool(name="zv", bufs=1) as zv_pool,
            tc.tile_pool(name="csb", bufs=2) as csb_pool,
            tc.tile_pool(name="ysb", bufs=2) as ysb_pool,
            tc.tile_pool(name="pacc", bufs=8, space="PSUM") as pacc_pool,
        ):
            # HAM warmup (PE clock gate opens after ~5us of matmul activity).
            wsrc = consts.tile([128, 128], f32, tag="wsrc")
            nc.vector.memset(wsrc[:], 1.0)
            warm_ps = pacc_pool.tile([128, 512], f32, tag="pacc")
            for _ in range(N_WARM):
                nc.tensor.matmul(
                    warm_ps[:, 0:128], wsrc[:], wsrc[:], start=True, stop=True
                )

            # Input DMAs in consumption order.
            bias_sb = consts.tile([O, 1], f32)
            wg_sbs = []
            zv_sbs = [[None] * 5 for _ in range(BPC)]
            wg0a_sb = consts.tile([128, 2, O], bf16, tag="wg0a")
            nc.sync.dma_start(wg0a_sb[:], wg0a_d[:])
            for p in range(5):
                wg_sb = consts.tile([128, NPAIR, O], bf16, tag=f"wg{p}")
                if p == 0:
                    nc.sync.dma_start(wg_sb[:, 2:, :], wg_ds[p][:, 2:, :])
                else:
                    nc.sync.dma_start(wg_sb[:], wg_ds[p][:])
                wg_sbs.append(wg_sb)
                zt = zv_pool.tile([128, ZC], bf16, tag=f"zv0_{p}")
                zv_sbs[0][p] = zt
                nc.sync.dma_start(zt[:, 0:ZSPLIT], zv_ds[p][0, :, 0:ZSPLIT])
                if p == 0:
                    nc.sync.dma_start(bias_sb[:], b_d[:])
            for p in range(5):
                nc.sync.dma_start(
                    zv_sbs[0][p][:, ZSPLIT:], zv_ds[p][0, :, ZSPLIT:]
                )
            for bb in range(1, BPC):
                for p in range(5):
                    zt = zv_pool.tile([128, ZC], bf16, tag=f"zv{bb}_{p}")
                    zv_sbs[bb][p] = zt
                    nc.sync.dma_start(zt[:, 0:ZSPLIT], zv_ds[p][bb, :, 0:ZSPLIT])
                for p in range(5):
                    nc.sync.dma_start(
                        zv_sbs[bb][p][:, ZSPLIT:], zv_ds[p][bb, :, ZSPLIT:]
                    )

            def emit_chunk(bb, t0, cw):
                M = []
                for p in range(5):
                    pacc = pacc_pool.tile([128, 512], f32, tag="pacc")
                    M.append(pacc)
                    for k in range(NPAIR):
                        off = 17 - 2 * k
                        lhs = (
                            wg0a_sb[:, k, :]
                            if (p == 0 and k < 2)
                            else wg_sbs[p][:, k, :]
                        )
                        nc.tensor.matmul(
                            pacc[:, 0:cw],
                            lhs,
                            zv_sbs[bb][p][:, t0 + off : t0 + off + cw],
                            start=(k == 0),
                            stop=(k == NPAIR - 1),
                        )
                    # DVE reads at most ONE PSUM operand per op: chain through
                    # SBUF intermediates, one M_p per pass.
                    if p == 1:
                        # t = M1 + bias
                        tt = csb_pool.tile([128, 512], f32, tag="t")
                        nc.vector.tensor_scalar_add(
                            tt[:, 0:cw], M[1][:, 0:cw], bias_sb[:]
                        )
                    if p == 2:
                        # t2 = M2 + t = M1 + M2 + bias
                        t2 = csb_pool.tile([128, 512], f32, tag="t2")
                        nc.vector.scalar_tensor_tensor(
                            t2[:, 0:cw], M[2][:, 0:cw], 0.0,
                            tt[:, 0:cw], Alu.add, Alu.add,
                        )
                    if p == 3:
                        # y0 = M0 + M3 + t2
                        s0 = csb_pool.tile([128, 512], f32, tag="s0")
                        nc.vector.scalar_tensor_tensor(
                            s0[:, 0:cw], M[0][:, 0:cw], 0.0,
                            t2[:, 0:cw], Alu.add, Alu.add,
                        )
                        y0 = ysb_pool.tile([128, 512], f32, tag="y0")
                        nc.vector.scalar_tensor_tensor(
                            y0[:, 0:cw], M[3][:, 0:cw], 0.0,
                            s0[:, 0:cw], Alu.add, Alu.add,
                        )
                        nc.sync.dma_start(
                            y_d[bb, 0, :, t0 : t0 + cw], y0[:, 0:cw]
                        )
                        # y1 = -2*M2 + t2 + 2*M3
                        u = csb_pool.tile([128, 512], f32, tag="u")
                        nc.vector.scalar_tensor_tensor(
                            u[:, 0:cw], M[2][:, 0:cw], -2.0,
                            t2[:, 0:cw], Alu.mult, Alu.add,
                        )
                        y1 = ysb_pool.tile([128, 512], f32, tag="y1")
                        nc.vector.scalar_tensor_tensor(
                            y1[:, 0:cw], M[3][:, 0:cw], 2.0,
                            u[:, 0:cw], Alu.mult, Alu.add,
                        )
                        nc.sync.dma_start(
                            y_d[bb, 1, :, t0 : t0 + cw], y1[:, 0:cw]
                        )
                        # v = 4*M3 + t2 (y2 minus M4)
                        vv = csb_pool.tile([128, 512], f32, tag="v")
                        nc.vector.scalar_tensor_tensor(
                            vv[:, 0:cw], M[3][:, 0:cw], 4.0,
                            t2[:, 0:cw], Alu.mult, Alu.add,
                        )
                # y2 = M4 + v — the only combine pass after the last matmul.
                y2 = ysb_pool.tile([128, 512], f32, tag="y2")
                nc.vector.scalar_tensor_tensor(
                    y2[:, 0:cw], M[4][:, 0:cw], 0.0,
                    vv[:, 0:cw], Alu.add, Alu.add,
                )
                nc.sync.dma_start(y_d[bb, 2, :, t0 : t0 + cw], y2[:, 0:cw])

            for bb in range(BPC):
                for t0, cw in CHUNKS:
                    emit_chunk(bb, t0, cw)
    nc.compile()
    return nc


_NC_CACHE = {}


def _get_program():
    if "nc" not in _NC_CACHE:
        _NC_CACHE["nc"] = _build()
    return _NC_CACHE["nc"]


def _prep_inputs(x, W, b):
    x = np.ascontiguousarray(x, dtype=np.float32)
    W = np.ascontiguousarray(W, dtype=np.float32)
    b = np.ascontiguousarray(b, dtype=np.float32)

    # --- data transform: v_p[c] = sum_s BT[p,s] x[3c + 2 + s] -------------
    # window origin for col c is 3c+2; c in [-16, 685] to cover tops/bottoms.
    xpad = np.zeros((B, T + 120, D), dtype=np.float32)
    xpad[:, 60 : 60 + T] = x
    cs = np.arange(-16, 686)
    idx = 60 + 3 * cs[None, :, None] + 2 + np.arange(5)[None, None, :]
    dwin = xpad[:, idx[0]]                       # [B, nc, 5, D]
    v = np.einsum(
        "ps,bcsd->bpcd", BT_MAT.astype(np.float32), dwin
    )                                            # [B, 5, nc, D]
    # zV[b, p, part, cc]: top v_p[cc-15], bottom v_p[cc-16]; cs[k]=c -> k=c+16
    zv = np.zeros((B, 5, 128, ZC), dtype=ml_dtypes.bfloat16)
    # cc in [0, 699]: top k = cc+1, bottom k = cc
    vt = v.transpose(0, 1, 3, 2)                 # [B, 5, D, nc]
    zv[:, :, :D, 0:700] = vt[:, :, :, 1:701]
    zv[:, :, D:, 0:700] = vt[:, :, :, 0:700]

    # --- weight transform -------------------------------------------------
    # U_{g,p}[d, o] = sum_i G[p,i] * W[o, (3g+2-i)*64 + d], tap >= 51 -> 0
    Wblk = W.reshape(O, NLAGS, D)
    wg = np.zeros((5, 128, NPAIR, O), dtype=np.float32)
    for p in range(5):
        for g in range(NG):
            U = np.zeros((D, O), dtype=np.float32)
            for i in range(3):
                tap = 3 * g + 2 - i
                if tap < NLAGS:
                    U += G_MAT[p, i].astype(np.float32) * Wblk[:, tap, :].T
            k, half = divmod(g, 2)
            wg[p, half * D : (half + 1) * D, k, :] = U
    wg_maps = {
        f"wg{p}": np.ascontiguousarray(wg[p]).astype(ml_dtypes.bfloat16)
        for p in range(5)
    }
    wg_maps["wg0a"] = np.ascontiguousarray(wg[0][:, 0:2, :]).astype(
        ml_dtypes.bfloat16
    )
    bvec = np.ascontiguousarray(b.reshape(O, 1))
    maps = []
    for c in range(N_CORES):
        m = {"bvec": bvec}
        for p in range(5):
            m[f"zv{p}"] = np.ascontiguousarray(
                zv[c * BPC : (c + 1) * BPC, p]
            )
        m.update(wg_maps)
        maps.append(m)
    return maps


def _assemble(res):
    # Per core: y_raw [BPC, 3, O, NT]; y[b, 3*tau+r, o] = y_raw[b, r, o, tau]
    outs = []
    for c in range(N_CORES):
        y_raw = res.results[c]["y"]
        y = (
            y_raw.transpose(0, 3, 1, 2)
            .reshape(BPC, NT * 3, O)[:, :T]
            .astype(np.float32)
        )
        outs.append(np.ascontiguousarray(y))
    return np.concatenate(outs, axis=0)


def kernel(x, W, b):
    in_maps = _prep_inputs(x, W, b)
    res = run_bass_kernel_spmd(
        _get_program(), in_maps, core_ids=list(range(N_CORES))
    )
    return _assemble(res)


def _ensure_ntff_hook():
    """The agent image's antenv lacks axon_hooks, so run_bass_kernel_spmd's
    trace path degrades to no-profile. Seed an equivalent module backed by
    the ctypes NTFF profiler from trn_agent_boot."""
    try:
        from antenv.axon_hooks import get_axon_ntff_profile_hook

        if get_axon_ntff_profile_hook() is not None:
            return True
    except ImportError:
        pass
    try:
        import types

        site_dir = "/root/.axon_site"
        if site_dir not in sys.path and os.path.isdir(site_dir):
            sys.path.insert(0, site_dir)
        from trn_agent_boot.trn_boot import _ntff_profile_via_ctypes

        hook = _ntff_profile_via_ctypes("/opt/axon/libaxon_pjrt.so")
        if hook is None:
            return False
        mod = types.ModuleType("antenv.axon_hooks")
        mod.get_axon_ntff_profile_hook = lambda: hook
        mod.set_axon_ntff_profile_hook = lambda h: None
        sys.modules["antenv.axon_hooks"] = mod
        import antenv

        antenv.axon_hooks = mod
        return True
    except Exception:
        return False


def kernel_traced(x, W, b, **kwargs):
    """Like kernel() but requests an NTFF trace; returns (y, BassKernelResults).

    Dev-loop only (test.py); the graded kernel() path never traces. The
    artifact upload is stubbed out since this container has no bucket access.
    """
    _ensure_ntff_hook()
    from concourse import bass_utils as _bu

    in_maps = _prep_inputs(x, W, b)
    orig_upload = _bu.upload_artifacts
    _bu.upload_artifacts = lambda tmpdir: f"local:{tmpdir}"
    try:
        res = run_bass_kernel_spmd(
            _get_program(), in_maps, core_ids=list(range(N_CORES)), trace=True, **kwargs
        )
    finally:
        _bu.upload_artifacts = orig_upload
    y = _assemble(res)
    return y, res
